# revision 25
# baseline (speedup 1.0000x reference)
"""DPTreeMultiheadAttention Trainium2 kernel.

Math reformulation: the reference scatters node keys into a [T,T] span
matrix, computes affinity, does a flipped-cumsum over rows + cumsum over
cols (containment DP) and gathers back at node positions.  That is exactly

    scores[b,h,q,n] = <q[b,h,q,:], sum_{m : span_m contained in span_n} k[b,h,m,:]>

i.e. scores = q @ (C_b @ k).T with a [Tk,Tk] 0/1 containment matrix
C_b[n,m] = (r_n <= r_m) & (c_m <= c_n) & (r_m <= c_m), computed on host
from the integer `indices` tensor.  Then softmax over nodes, attn = w @ v,
and the out-projection.  Verified exact vs the reference (rel err ~1e-6
in fp32).

Precision: fp16 matmul operands everywhere (PE runs fp16 at full rate —
1 cycle/row — while plain fp32 is 4x slower; fp16's 11-bit mantissa keeps
end-to-end error at ~1.2e-3 of the output absmax, measured).  PSUM
accumulation is fp32.  All values fit fp16 range comfortably except
exp(scores) (up to e^21), which is staged in fp32 and only cast to fp16
after normalization (weights <= 1).

Softmax skips the running-max shift: logits for this problem are ~+-21
and exp() stays comfortably inside fp32 range (overflow needs >88).

All per-head post-matmul work (PSUM evacuation, softmax normalize,
weight-transpose copies) is fused across the core's 4 heads into wide
instructions; per-head matmuls write disjoint 256/128-column slices of
shared PSUM tiles.  Inputs are shipped as merged [input | weight] DMA
groups, split into contraction-chunk pieces and ordered so the score-path
chain (k/q projections -> containment matmul -> scores) starts as early
as possible while v/out-projection weights stream in behind it.

Sharding: 8 cores = 4 batches x 2 head-halves (4 heads = 512 features
each).  Each core projects q/k/v for its (batch, head-half), does the
containment matmul, attention, and a partial out-projection over its 512
features.  Host sums the two partial out-projections per batch.
"""

import os
import sys

for _p in ("/opt/trn_rl_repo", "/root/.axon_site/_ro/trn_rl_repo"):
    if os.path.isdir(_p) and _p not in sys.path:
        sys.path.append(_p)

import numpy as np

import concourse.bacc as bacc
import concourse.mybir as mybir
import concourse.tile as tile
from concourse import masks
from concourse.bass_utils import run_bass_kernel_spmd

F16 = np.float16

T = 128          # leaf sequence length
TK = 255         # tree nodes
TKP = 256        # padded nodes
B = 4            # batch
H = 8            # heads
D = 128          # head dim
E = 1024         # embed dim
LQ = 128         # query length
NH = 4           # heads per core
F = NH * D       # features per core (512)
N_CORES = 8

_CACHE = {}


def _build_program(repeat=1):
    nc = bacc.Bacc("TRN2", target_bir_lowering=False, debug=False)
    f32 = mybir.dt.float32
    f16 = mybir.dt.float16

    def din(name, shape):
        return nc.dram_tensor(name, shape, f16, kind="ExternalInput").ap()

    # merged input groups (all fp16):
    kg_d = din("kg", [E, TKP + F])      # [kT | wkT]
    qg_d = din("qg", [E, LQ + F])       # [qT | wqT]
    vg_d = din("vg", [E, TKP + F])      # [vT | wvT]
    bias_d = din("bias", [3, F])        # bq*scale, bk, bv
    ct_d = din("CT", [TKP, TKP])        # containment [m, n], row/col 255 = 0
    wo1_d = din("wo1", [F, E])          # out_proj[:, hs].T
    out_shape = [LQ, E] if repeat == 1 else [repeat, LQ, E]
    out_d = nc.dram_tensor("out", out_shape, f16, kind="ExternalOutput").ap()

    with tile.TileContext(nc) as tc:
        with (
            tc.tile_pool(name="hold", bufs=1) as hp,
            tc.tile_pool(name="sm", bufs=1) as smp,
            tc.tile_pool(name="ps", bufs=1, space="PSUM") as psp,
        ):
          for _rep in range(repeat):
            # ---- persistent SBUF tiles + loads (order = priority) ----
            kg_sb = hp.tile([128, 8, TKP + F], f16, tag="kg_sb")
            qg_sb = hp.tile([128, 8, LQ + F], f16, tag="qg_sb")
            vg_sb = hp.tile([128, 8, TKP + F], f16, tag="vg_sb")
            ct_sb = hp.tile([128, 2, TKP], f16, tag="ct_sb")
            wo_sb = hp.tile([128, 4, E], f16, tag="wo_sb")
            b_sb = hp.tile([1, 3, F], f16, tag="b_sb")
            ones_sb = hp.tile([1, 128], f16, tag="ones_sb")
            identh = hp.tile([128, 128], f16, tag="identh")

            kg_r = kg_d.rearrange("(a p) m -> p a m", p=128)
            for c0 in range(0, 8, 2):
                nc.sync.dma_start(kg_sb[:, c0 : c0 + 2, :], kg_r[:, c0 : c0 + 2, :])
            nc.sync.dma_start(b_sb[:], bias_d.rearrange("(o w) f -> o w f", o=1))
            nc.sync.dma_start(ct_sb[:], ct_d.rearrange("(a p) n -> p a n", p=128))
            qg_r = qg_d.rearrange("(a p) l -> p a l", p=128)
            for c0 in range(0, 8, 2):
                nc.sync.dma_start(qg_sb[:, c0 : c0 + 2, :], qg_r[:, c0 : c0 + 2, :])
            vg_r = vg_d.rearrange("(a p) m -> p a m", p=128)
            nc.sync.dma_start(vg_sb[:, 0:4, :], vg_r[:, 0:4, :])
            nc.sync.dma_start(vg_sb[:, 4:8, :], vg_r[:, 4:8, :])
            nc.sync.dma_start(wo_sb[:], wo1_d.rearrange("(a p) e -> p a e", p=128))
            nc.vector.memset(ones_sb[:], 1.0)
            masks.make_identity(nc, identh[:])

            # ---- k projection: kp[m, f] ----
            kp_sb = hp.tile([128, 2, F], f16, tag="kp_sb")
            for mi in range(2):
                ps = psp.tile([128, F], f32, tag="mm", bufs=3)
                for a in range(8):
                    nc.tensor.matmul(
                        ps[:],
                        kg_sb[:, a, mi * 128 : (mi + 1) * 128],
                        kg_sb[:, a, TKP : TKP + F],
                        start=(a == 0), stop=False,
                    )
                nc.tensor.matmul(ps[:], ones_sb[:1, :], b_sb[:1, 1, :],
                                 start=False, stop=True)
                nc.scalar.copy(kp_sb[:, mi, :], ps[:])

            # ---- q projection, directly per-head transposed: qt[d, l]
            # (weights as stationary operand; skips the qp round-trip) ----
            qt_sb = hp.tile([128, NH, LQ], f16, tag="qt_sb")
            psq = psp.tile([128, NH, LQ], f32, tag="p1", bufs=3)
            for h in range(NH):
                hsl = slice(LQ + h * D, LQ + (h + 1) * D)
                for a in range(8):
                    nc.tensor.matmul(psq[:, h, :], qg_sb[:, a, hsl],
                                     qg_sb[:, a, 0:LQ],
                                     start=(a == 0), stop=False)
                nc.tensor.matmul(psq[:, h, :],
                                 b_sb[:1, 0, h * D : (h + 1) * D],
                                 ones_sb[:1, :], start=False, stop=True)
            nc.scalar.copy(qt_sb[:], psq[:])

            # ---- K_agg.T: Kagg[d, n] packed over heads ----
            kagg_sb = hp.tile([128, NH, TKP], f16, tag="kagg_sb")
            psk = psp.tile([128, NH, TKP], f32, tag="p2")
            for h in range(NH):
                hsl = slice(h * D, (h + 1) * D)
                nc.tensor.matmul(psk[:, h, :], kp_sb[:, 0, hsl], ct_sb[:, 0, :],
                                 start=True, stop=False)
                nc.tensor.matmul(psk[:, h, :], kp_sb[:, 1, hsl], ct_sb[:, 1, :],
                                 start=False, stop=True)
            nc.scalar.copy(kagg_sb[:], psk[:])

            # ---- scores packed over heads ----
            pss = psp.tile([128, NH, TKP], f32, tag="p2")
            for h in range(NH):
                nc.tensor.matmul(pss[:, h, :], qt_sb[:, h, :], kagg_sb[:, h, :],
                                 start=True, stop=True)

            # ---- v projection — fills PE idle while softmax runs ----
            vp_sb = hp.tile([128, 2, F], f16, tag="vp_sb")
            for mi in range(2):
                ps = psp.tile([128, F], f32, tag="mm", bufs=3)
                for a in range(8):
                    nc.tensor.matmul(
                        ps[:], vg_sb[:, a, mi * 128 : (mi + 1) * 128],
                        vg_sb[:, a, TKP : TKP + F],
                        start=(a == 0), stop=False,
                    )
                nc.tensor.matmul(ps[:], ones_sb[:1, :], b_sb[:1, 2, :],
                                 start=False, stop=True)
                nc.scalar.copy(vp_sb[:, mi, :], ps[:])

            # ---- softmax over nodes (no max shift; logits ~ +-21).
            # exp stays fp32 (e^21 overflows fp16); normalized w <= 1 is
            # cast to fp16 by the normalize multiply. ----
            wexp = smp.tile([128, NH, TKP], f32, tag="wexp")
            ssum = smp.tile([128, NH], f32, tag="ssum")
            for h in range(NH):
                nc.scalar.activation(
                    wexp[:, h, :TK], pss[:, h, :TK],
                    mybir.ActivationFunctionType.Exp,
                    accum_out=ssum[:, h : h + 1],
                )
            rinv = smp.tile([128, NH], f32, tag="rinv")
            nc.vector.reciprocal(rinv[:], ssum[:])
            wgt = smp.tile([128, NH, TKP], f16, tag="wgt")
            nc.vector.tensor_mul(wgt[:, :, :TK], wexp[:, :, :TK],
                                 rinv[:].to_broadcast([128, NH, TK]))

            # ---- w.T via PE transposes (packed), then attn.T ----
            pt0 = psp.tile([128, NH, 128], f16, tag="p1", bufs=3)
            pt1 = psp.tile([127, NH, 128], f16, tag="p1", bufs=3)
            wt0 = smp.tile([128, NH, 128], f16, tag="wt0")
            wt1 = smp.tile([127, NH, 128], f16, tag="wt1")
            for h in range(NH):
                nc.tensor.transpose(pt0[:, h, :], wgt[:, h, 0:128], identh[:])
                nc.tensor.transpose(pt1[:, h, :], wgt[:, h, 128:TK], identh[:])
            nc.vector.tensor_copy(wt0[:], pt0[:])
            nc.vector.tensor_copy(wt1[:], pt1[:])

            at_sb = hp.tile([128, NH, LQ], f16, tag="at_sb")
            psa = psp.tile([128, NH, LQ], f32, tag="p1", bufs=3)
            for h in range(NH):
                hsl = slice(h * D, (h + 1) * D)
                nc.tensor.matmul(psa[:, h, :], vp_sb[:, 0, hsl], wt0[:, h, :],
                                 start=True, stop=False)
                nc.tensor.matmul(psa[:, h, :], vp_sb[0:127, 1, hsl], wt1[:, h, :],
                                 start=False, stop=True)
            nc.vector.tensor_copy(at_sb[:], psa[:])

            # ---- partial out-projection ----
            out_sb = hp.tile([128, E], f16, tag="out_sb")
            for eo in range(2):
                ps = psp.tile([128, 512], f32, tag="mm", bufs=3)
                for h in range(NH):
                    nc.tensor.matmul(
                        ps[:], at_sb[:, h, :],
                        wo_sb[:, h, eo * 512 : (eo + 1) * 512],
                        start=(h == 0), stop=(h == 3),
                    )
                cp = nc.scalar.copy if eo == 0 else nc.vector.tensor_copy
                cp(out_sb[:, eo * 512 : (eo + 1) * 512], ps[:])
                od = out_d if repeat == 1 else out_d[_rep]
                nc.sync.dma_start(od[:, eo * 512 : (eo + 1) * 512],
                                  out_sb[:, eo * 512 : (eo + 1) * 512])

    nc.compile()
    return nc


def _get_program():
    if "nc" not in _CACHE:
        _CACHE["nc"] = _build_program()
    return _CACHE["nc"]


def _prep_inputs(query, key, value, indices, in_proj_weight, in_proj_bias,
                 out_proj_weight):
    scale = float(D) ** -0.5
    wq, wk, wv = (in_proj_weight[0:E], in_proj_weight[E:2 * E],
                  in_proj_weight[2 * E:3 * E])
    bq, bk, bv = (in_proj_bias[0:E], in_proj_bias[E:2 * E],
                  in_proj_bias[2 * E:3 * E])

    r = indices[:, :, 0].astype(np.int64)
    c = indices[:, :, 1].astype(np.int64)
    # ct[b][m, n] = 1 iff span_m is contained in span_n (and m valid triu)
    ct = (
        (r[:, None, :] <= r[:, :, None])
        & (c[:, :, None] <= c[:, None, :])
        & (r[:, :, None] <= c[:, :, None])
    ).astype(F16)  # [B, m, n]

    in_maps = []
    for core in range(N_CORES):
        b = core // 2
        hh = core % 2
        hs = slice(hh * F, (hh + 1) * F)

        kg = np.zeros((E, TKP + F), F16)
        kg[:, :TK] = key[:, b, :].T
        kg[:, TKP:] = wk[hs].T
        qg = np.empty((E, LQ + F), F16)
        qg[:, :LQ] = query[:, b, :].T
        qg[:, LQ:] = (wq[hs] * scale).T
        vg = np.zeros((E, TKP + F), F16)
        vg[:, :TK] = value[:, b, :].T
        vg[:, TKP:] = wv[hs].T
        ctp = np.zeros((TKP, TKP), F16)
        ctp[:TK, :TK] = ct[b]

        in_maps.append({
            "kg": kg,
            "qg": qg,
            "vg": vg,
            "bias": np.ascontiguousarray(
                np.stack([bq[hs] * scale, bk[hs], bv[hs]]).astype(F16)),
            "CT": ctp,
            "wo1": np.ascontiguousarray(out_proj_weight[:, hs].T).astype(F16),
        })
    return in_maps


def kernel(query, key, value, indices, in_proj_weight, in_proj_bias,
           out_proj_weight, out_proj_bias, _run_kwargs=None):
    query = np.asarray(query, np.float32)
    key = np.asarray(key, np.float32)
    value = np.asarray(value, np.float32)
    indices = np.asarray(indices)
    in_proj_weight = np.asarray(in_proj_weight, np.float32)
    in_proj_bias = np.asarray(in_proj_bias, np.float32)
    out_proj_weight = np.asarray(out_proj_weight, np.float32)
    out_proj_bias = np.asarray(out_proj_bias, np.float32)

    in_maps = _prep_inputs(query, key, value, indices, in_proj_weight,
                           in_proj_bias, out_proj_weight)
    nc = _get_program()
    res = run_bass_kernel_spmd(
        nc, in_maps, core_ids=list(range(N_CORES)), **(_run_kwargs or {})
    )
    if _run_kwargs:
        _CACHE["last_results"] = res
    parts = [res.results[i]["out"].astype(np.float32) for i in range(N_CORES)]
    out = np.empty((LQ, B, E), np.float32)
    for b in range(B):
        out[:, b, :] = parts[2 * b] + parts[2 * b + 1] + out_proj_bias
    return out
